# revision 70
# baseline (speedup 1.0000x reference)
"""BiLinearInteraction Trainium2 kernel (8 NeuronCores, data-parallel over batch).

Reference computation (per pair p=(i,j) of F=26 fields, P=325 pairs):
    out[b, p*64:(p+1)*64] = (x[i, b, :] @ W[p]) * x[j, b, :]
Full shapes: x [26, 4096, 64] f32, W [325, 64, 64] f32 -> out [4096, 20800] f32.

Strategy (v2)
- Shard batch 4096 -> 8 x 512 (4 batch tiles of 128 rows per core), replicate W.
- Parity-packed operands: even fields' matmul data on SBUF partitions 0-63,
  odd fields' on 64-127 (PE row groups run concurrently via tile_position),
  with NO duplication -> 6.1 MB/core of input HBM traffic (was 10.4).
- All input loads ride the ACT HWDGE ring, output writes the SP HWDGE ring;
  GPSIMD does no DMA descriptor work and is free to run elementwise muls.
- Per batch tile, matmuls accumulate into [128, <=2048] PSUM group tiles
  (4 banks, 2 bufs = whole PSUM). Groups are classed V/A/G:
    V: DVE multiplies straight out of PSUM (fp32, 1x) into the bf16 stage.
    A: one big ACT copy drains the group PSUM -> stage (bf16), then DVE
       multiplies in place at 2x (all-bf16 packed mode).
    G: same drain, but GPSIMD does the in-place mul.
  This spreads the 10.65M-elem/core evacuate+multiply load across all three
  engines (~60us each) instead of DVE+ACT only (~95us DVE in v1).
- Output staged per half batch-tile and written as two ~2.5MB DMAs per tile
  (bf16; host upcasts) -> ~385 GB/s ring efficiency vs ~340 at 0.8MB chunks.
"""

import os
import sys

sys.path.insert(0, "/opt/trn_rl_repo")

from itertools import combinations

import ml_dtypes
import numpy as np

import concourse.bass as bass
import concourse.mybir as mybir
from concourse import bacc
from concourse.tile import TileContext

F, D, B = 26, 64, 4096
NCORES = 8
BC = B // NCORES          # 512 batch rows per core
NT = BC // 128            # 4 batch tiles of 128 rows
PAIRS = list(combinations(range(F), 2))
P = len(PAIRS)            # 325
OUT_COLS = P * D          # 20800

N_PAIRS = [F - 1 - i for i in range(F - 1)]           # pairs with left field i
P_START = [sum(N_PAIRS[:i]) for i in range(F - 1)]    # first pair index of field i
FIELD_START = [P_START[i] * D for i in range(F - 1)]  # output col where field i begins
FIELD_END = [FIELD_START[i] + N_PAIRS[i] * D for i in range(F - 1)]

XNW = F * D               # xn cols per batch tile                           = 1664

# PSUM group grid per batch tile: (c0, c1, class).  FIELD-ALIGNED groups of
# <=2048 f32 cols (one pool slot = 4 banks; 2 bufs = whole PSUM).
#   V: DVE mul straight from PSUM (fp32, 1x) -- no ACT involvement.
#   A: ACT drain (f32 PSUM -> bf16 stage) + DVE in-place mul at 2x.
# GPSIMD is NOT used for muls: measured on HW, GpSimd tensor_tensor and DVE
# tensor_tensor serialize on the shared SBUF port pair (the blocked op stalls
# for the other's full duration), so GPSIMD adds no elementwise throughput.
# V groups are interleaved between A groups so DVE and ACT stay concurrently
# busy through the tile; ~21% of cols are V, balancing DVE (1x V muls + 2x A
# muls) against ACT (1x drains).
# The PE row group of all matmuls in a group is the GROUP index parity: two
# concurrent (different-row-group) matmuls writing the same PSUM bank is a
# fatal HW collision.  Same-group pieces (which may share a bank at interior
# field boundaries) share a row group -> serial -> safe; adjacent groups
# alternate -> dual-row-group concurrency across groups (separate pool slots
# -> disjoint banks).  Field-aligned groups mean each field has exactly one
# parity, so xt needs no duplication (1.7MB instead of 3.4).
GROUPS = [
    (0, 2048, 'A'), (2048, 4096, 'V'), (4096, 6144, 'A'),
    (6144, 8192, 'A'), (8192, 9856, 'A'),
    (9856, 11904, 'A'), (11904, 13952, 'V'), (13952, 16000, 'A'),
    (16000, 18048, 'A'), (18048, 20096, 'A'), (20096, 20800, 'V'),
]
HALF = 9856               # st0 covers cols [0, 9856), st1 [9856, 20800)

# Contiguous class runs (mul granularity: field pieces within a run).
RUNS = []
for (_g0, _g1, _cls) in GROUPS:
    if RUNS and RUNS[-1][2] == _cls and RUNS[-1][1] == _g0:
        RUNS[-1] = (RUNS[-1][0], _g1, _cls)
    else:
        RUNS.append((_g0, _g1, _cls))

# PSUM 512-col blocks (bank-aligned within each group's psum tile), carrying
# the group parity and the running per-parity W pack offset.  W's top half
# (partitions 0-63) holds even groups' columns in order, bottom half odd
# groups'.  Each field belongs to one group, hence one parity -> PAR[i].
# Block parity: the PE row group of every matmul is its 512-block's global
# index parity.  Same-bank pieces (same block) share a row group -> serial ->
# safe; adjacent blocks alternate -> dual-row-group concurrency everywhere,
# including inside large fields.  Costs xt duplication (all fields in both
# halves), which measured faster than field-parity variants that save the
# bytes but serialize each field's matmul stream.
BLOCKS = []               # (c0, c1, parity, w_off)
_offs = [0, 0]
_idx = 0
for (_g0, _g1, _cls) in GROUPS:
    _c = _g0
    while _c < _g1:
        _c1 = min(_c + 512, _g1)
        _par = _idx % 2
        BLOCKS.append((_c, _c1, _par, _offs[_par]))
        _offs[_par] += _c1 - _c
        _idx += 1
        _c = _c1
W_COLS = max(_offs)       # top half; the shorter half is padded
# W load chunk boundaries (w-offset space).  Finer early chunks let the
# first matmuls start ~6us in instead of ~12.
W_BOUNDS = [0, 1024, 3072, 6144, W_COLS]
# Output cols where a 512-block straddles a W chunk boundary in w-offset
# space: matmul pieces must split there so each rhs lives in one load tile.
W_EXTRA_SPLITS = []
for (_c0, _c1, _par, _boff) in BLOCKS:
    for _s in W_BOUNDS[1:-1]:
        if _boff < _s < _boff + (_c1 - _c0):
            W_EXTRA_SPLITS.append(_c0 + (_s - _boff))

# xt pack: every field in both halves (any block parity can use any field).
XT_OFF = {i: i * 128 for i in range(F)}
XTW = F * 128             # xt cols per batch tile (per half) = 3328


def _block_of(col):
    for b in BLOCKS:
        if b[0] <= col < b[1]:
            return b
    raise ValueError(col)

F32 = mybir.dt.float32
BF16 = mybir.dt.bfloat16


def _even_splits(c0, c1, n):
    step = -(-((c1 - c0) // n) // 64) * 64
    step = max(step, 64)
    out = []
    c = c0
    while c < c1:
        out.append((c, min(c + step, c1)))
        c += step
    return out


def _field_of(col):
    for i in range(F - 1):
        if FIELD_START[i] <= col < FIELD_END[i]:
            return i
    raise ValueError(col)


def _pieces(c0, c1, extra=()):
    """Split [c0, c1) at field starts and any extra boundaries.
    Returns list of (p0, p1, field)."""
    bounds = {c0, c1}
    bounds.update(s for s in FIELD_START if c0 < s < c1)
    bounds.update(e for e in extra if c0 < e < c1)
    bs = sorted(bounds)
    return [(a, b, _field_of(a)) for a, b in zip(bs, bs[1:])]


def _mm_pieces(g0, g1):
    """Matmul pieces: additionally split at 512-col PSUM bank boundaries
    (relative to the group base = block boundaries) and at w-load splits."""
    extra = set(range(g0 + 512, g1, 512))
    extra.update(W_EXTRA_SPLITS)
    return _pieces(g0, g1, extra)


def build_bass() -> bass.Bass:
    # Bisection flags (default off = full-featured kernel).
    no_gps = os.environ.get("K_NO_GPS", "0") == "1"       # gpsimd muls -> DVE
    no_inplace = os.environ.get("K_NO_INPLACE", "0") == "1"  # muls via cp tile
    swdge_loads = os.environ.get("K_SWDGE_LOADS", "0") == "1"  # loads on gpsimd
    wsplit = int(os.environ.get("K_WRITE_SPLIT", "2"))    # write DMAs per half
    nt_limit = int(os.environ.get("K_NT_LIMIT", str(NT)))  # batch tiles to run
    ngroups = int(os.environ.get("K_NGROUPS", str(len(GROUPS))))
    no_muls = os.environ.get("K_NO_MULS", "0") == "1"
    no_drains = os.environ.get("K_NO_DRAINS", "0") == "1"
    nc = bacc.Bacc()
    xn = nc.declare_dram_parameter("xn", [128, NT * XNW], BF16, isOutput=False)
    xt = nc.declare_dram_parameter("xt", [128, NT * XTW], BF16, isOutput=False)
    w = nc.declare_dram_parameter("w", [128, W_COLS], BF16, isOutput=False)
    out = nc.declare_dram_parameter("out", [BC, OUT_COLS], BF16, isOutput=True)

    with TileContext(nc) as tc:
        with (
            tc.tile_pool(name="consts", bufs=1) as consts,
            tc.tile_pool(name="stage", bufs=4) as stage_pool,
            tc.tile_pool(name="cp", bufs=2) as cp_pool,
            tc.tile_pool(name="psum", bufs=2, space="PSUM") as psum_pool,
        ):
            # Separate tile objects per load DMA keep dependency granularity
            # at the piece level.  Loads ride the SP HWDGE ring (its preamble
            # clears earliest), ordered by first use: tiny first chunks get
            # the first matmul going ~6us in; the output writes queue behind
            # but only start ~20us in, when all loads have drained.
            w_t = [consts.tile([128, b - a], BF16, tag=f"w{k}", name=f"w{k}")
                   for k, (a, b) in enumerate(zip(W_BOUNDS, W_BOUNDS[1:]))]
            xt00 = consts.tile([128, 256], BF16, tag="xt00", name="xt00")
            xt0r = consts.tile([128, XTW - 256], BF16, tag="xt0r", name="xt0r")
            xtr = consts.tile([128, (NT - 1) * XTW], BF16, tag="xtr", name="xtr")
            xn0 = consts.tile([128, XNW], BF16, tag="xn0", name="xn0")
            xnr = consts.tile([128, (NT - 1) * XNW], BF16, tag="xnr", name="xnr")

            # All loads on the SP ring in first-use order: the FIFO ring
            # drains them in priority order, so the critical early chunks
            # never share bandwidth with the later bulk loads.  (Spreading
            # issues across rings measured 20us WORSE -- the rings
            # round-robin at packet granularity, starving the early chunks.)
            ldeng = nc.sync if not swdge_loads else nc.gpsimd
            ldeng.dma_start(out=xt00[:], in_=xt[:, 0:256])
            ldeng.dma_start(out=w_t[0][:], in_=w[:, W_BOUNDS[0]:W_BOUNDS[1]])
            ldeng.dma_start(out=xn0[:], in_=xn[:, 0:XNW])
            ldeng.dma_start(out=xt0r[:], in_=xt[:, 256:XTW])
            ldeng.dma_start(out=w_t[1][:], in_=w[:, W_BOUNDS[1]:W_BOUNDS[2]])
            ldeng.dma_start(out=w_t[2][:], in_=w[:, W_BOUNDS[2]:W_BOUNDS[3]])
            ldeng.dma_start(out=w_t[3][:], in_=w[:, W_BOUNDS[3]:W_BOUNDS[4]])
            ldeng.dma_start(out=xtr[:], in_=xt[:, XTW:NT * XTW])
            ldeng.dma_start(out=xnr[:], in_=xn[:, XNW:NT * XNW])

            def xt_slice(t, i, r0):
                c = XT_OFF[i]
                if t == 0:
                    if c + 128 <= 256:
                        return xt00[r0:r0 + D, c:c + 128]
                    return xt0r[r0:r0 + D, c - 256:c - 256 + 128]
                c += (t - 1) * XTW
                return xtr[r0:r0 + D, c:c + 128]

            def w_slice(c0, c1):
                b0, b1, par, boff = _block_of(c0)
                assert c1 <= b1, (c0, c1, b0, b1)
                r0 = par * D
                wc = boff + (c0 - b0)
                n = c1 - c0
                for k in range(len(W_BOUNDS) - 1):
                    if wc + n <= W_BOUNDS[k + 1]:
                        assert wc >= W_BOUNDS[k], (c0, c1, wc)
                        wk = wc - W_BOUNDS[k]
                        return r0, w_t[k][r0:r0 + D, wk:wk + n]
                raise AssertionError((c0, c1, wc))

            def xn_slice(t, i, c0, c1):
                c = (i + 1) * D + (c0 - FIELD_START[i])
                if t > 0:
                    c += (t - 1) * XNW
                src = xn0 if t == 0 else xnr
                return src[:, c:c + (c1 - c0)]

            for t in range(nt_limit):
                st0 = stage_pool.tile([128, HALF], BF16, tag="stage",
                                      name=f"st{t}a")
                st1 = stage_pool.tile([128, OUT_COLS - HALF], BF16, tag="stage",
                                      name=f"st{t}b")

                def st_slice(c0, c1):
                    if c0 >= HALF:
                        return st1[:, c0 - HALF:c1 - HALF]
                    assert c1 <= HALF
                    return st0[:, c0:c1]

                if no_muls:
                    nc.vector.memset(st0[:], 0.0)
                    nc.vector.memset(st1[:], 0.0)

                # A/G-run mul pieces not yet emitted, per run index.
                pending = {}
                cp_tiles = {}
                for ri, (r0_, r1_, rcls) in enumerate(RUNS):
                    if rcls != 'V':
                        pending[ri] = _pieces(r0_, r1_)
                        if no_inplace:
                            cp_tiles[ri] = cp_pool.tile(
                                [128, r1_ - r0_], BF16, tag="cp",
                                name=f"cp{t}_{ri}")

                gskip = int(os.environ.get("K_GSKIP", "0"))
                mm_filter = os.environ.get("K_MM_FILTER")
                if mm_filter is not None:
                    mm_filter = {int(v) for v in mm_filter.split(",")}
                mm_idx = 0
                for (g0, g1, gcls) in GROUPS[gskip:ngroups]:
                    ps = psum_pool.tile([128, g1 - g0], F32, tag="ps",
                                        name=f"ps{t}_{g0}")
                    for (c0, c1, i) in _mm_pieces(g0, g1):
                        mm_idx += 1
                        if mm_filter is not None and (mm_idx - 1) not in mm_filter:
                            continue
                        r0, rhs = w_slice(c0, c1)
                        nc.tensor.matmul(
                            ps[:, c0 - g0:c1 - g0],
                            xt_slice(t, i, r0),
                            rhs,
                            start=True, stop=True,
                        )
                    if gcls == 'V':
                        for (c0, c1, i) in _pieces(g0, g1):
                            if no_muls:
                                break
                            nc.vector.tensor_mul(
                                st_slice(c0, c1),
                                ps[:, c0 - g0:c1 - g0],
                                xn_slice(t, i, c0, c1),
                            )
                    else:
                        ri = next(k for k, (a, b, cl) in enumerate(RUNS)
                                  if a <= g0 < b)
                        run0 = RUNS[ri][0]
                        # One big ACT drain (f32 PSUM -> bf16, cast).
                        if no_inplace:
                            drain_dst = cp_tiles[ri][:, g0 - run0:g1 - run0]
                        else:
                            drain_dst = st_slice(g0, g1)
                        if not no_drains:
                            nc.scalar.copy(out=drain_dst, in_=ps[:])
                        # Emit muls for run pieces fully drained now.
                        eng = nc.vector if (gcls == 'A' or no_gps) else nc.gpsimd
                        done = [pc for pc in pending[ri] if pc[1] <= g1]
                        for (c0, c1, i) in done:
                            pending[ri].remove((c0, c1, i))
                            if no_muls:
                                continue
                            if no_inplace:
                                msrc = cp_tiles[ri][:, c0 - run0:c1 - run0]
                            else:
                                msrc = st_slice(c0, c1)
                            eng.tensor_mul(
                                st_slice(c0, c1),
                                msrc,
                                xn_slice(t, i, c0, c1),
                            )
                    if g1 == HALF:
                        for (a, b) in _even_splits(0, HALF, wsplit):
                            nc.sync.dma_start(
                                out=out[t * 128:(t + 1) * 128, a:b],
                                in_=st0[:, a:b],
                            )
                if ngroups >= len(GROUPS):
                    assert all(not v for v in pending.values())
                    # Last tile: finer writes so the final transfer starts
                    # as early as possible (it is the kernel's tail).
                    ws = wsplit * 2 if t == NT - 1 else wsplit
                    for (a, b) in _even_splits(HALF, OUT_COLS, ws):
                        nc.sync.dma_start(
                            out=out[t * 128:(t + 1) * 128, a:b],
                            in_=st1[:, a - HALF:b - HALF],
                        )
    nc.compile()
    return nc


def prep_inputs(x: np.ndarray, W: np.ndarray):
    """Full inputs -> per-core in_maps with block-parity-packed bf16 layouts."""
    x = np.ascontiguousarray(np.asarray(x, dtype=np.float32))
    W = np.ascontiguousarray(np.asarray(W, dtype=np.float32))
    wg = W.transpose(1, 0, 2).reshape(D, OUT_COLS)
    w_top = np.zeros((D, W_COLS), np.float32)
    w_bot = np.zeros((D, W_COLS), np.float32)
    for (c0, c1, par, boff) in BLOCKS:
        dst = w_top if par == 0 else w_bot
        dst[:, boff:boff + (c1 - c0)] = wg[:, c0:c1]
    w_p = np.ascontiguousarray(
        np.concatenate([w_top, w_bot], axis=0).astype(ml_dtypes.bfloat16)
    )
    in_maps = []
    for c in range(NCORES):
        xc = x[:, c * BC:(c + 1) * BC, :]                       # [26, 512, 64]
        xn_p = np.ascontiguousarray(
            xc.reshape(F, NT, 128, D).transpose(2, 1, 0, 3)
            .reshape(128, NT * XNW).astype(ml_dtypes.bfloat16)
        )
        # xtd[d, t*XTW + f*128 + r] = xc[f, t*128+r, d], duplicated to both
        # partition halves so any block parity can use any field.
        xtd = (xc.reshape(F, NT, 128, D).transpose(3, 1, 0, 2)
               .reshape(D, NT * XTW))
        xt_p = np.ascontiguousarray(
            np.concatenate([xtd, xtd], axis=0).astype(ml_dtypes.bfloat16)
        )
        in_maps.append({"xn": xn_p, "xt": xt_p, "w": w_p})
    return in_maps


_CACHED_NC = None


def kernel(x: np.ndarray, W: np.ndarray) -> np.ndarray:
    global _CACHED_NC
    from concourse.bass_utils import run_bass_kernel_spmd

    if _CACHED_NC is None:
        _CACHED_NC = build_bass()
    in_maps = prep_inputs(x, W)
    res = run_bass_kernel_spmd(_CACHED_NC, in_maps, list(range(NCORES)))
    shards = [
        np.asarray(res.results[c]["out"]).astype(np.float32) for c in range(NCORES)
    ]
    return np.concatenate(shards, axis=0)


# revision 71
# speedup vs baseline: 1.0209x; 1.0209x over previous
"""BiLinearInteraction Trainium2 kernel (8 NeuronCores, data-parallel over batch).

Reference computation (per pair p=(i,j) of F=26 fields, P=325 pairs):
    out[b, p*64:(p+1)*64] = (x[i, b, :] @ W[p]) * x[j, b, :]
Full shapes: x [26, 4096, 64] f32, W [325, 64, 64] f32 -> out [4096, 20800] f32.

Strategy (v2)
- Shard batch 4096 -> 8 x 512 (4 batch tiles of 128 rows per core), replicate W.
- Parity-packed operands: even fields' matmul data on SBUF partitions 0-63,
  odd fields' on 64-127 (PE row groups run concurrently via tile_position),
  with NO duplication -> 6.1 MB/core of input HBM traffic (was 10.4).
- All input loads ride the ACT HWDGE ring, output writes the SP HWDGE ring;
  GPSIMD does no DMA descriptor work and is free to run elementwise muls.
- Per batch tile, matmuls accumulate into [128, <=2048] PSUM group tiles
  (4 banks, 2 bufs = whole PSUM). Groups are classed V/A/G:
    V: DVE multiplies straight out of PSUM (fp32, 1x) into the bf16 stage.
    A: one big ACT copy drains the group PSUM -> stage (bf16), then DVE
       multiplies in place at 2x (all-bf16 packed mode).
    G: same drain, but GPSIMD does the in-place mul.
  This spreads the 10.65M-elem/core evacuate+multiply load across all three
  engines (~60us each) instead of DVE+ACT only (~95us DVE in v1).
- Output staged per half batch-tile and written as two ~2.5MB DMAs per tile
  (bf16; host upcasts) -> ~385 GB/s ring efficiency vs ~340 at 0.8MB chunks.
"""

import os
import sys

sys.path.insert(0, "/opt/trn_rl_repo")

from itertools import combinations

import ml_dtypes
import numpy as np

import concourse.bass as bass
import concourse.mybir as mybir
from concourse import bacc
from concourse.tile import TileContext

F, D, B = 26, 64, 4096
NCORES = 8
BC = B // NCORES          # 512 batch rows per core
NT = BC // 128            # 4 batch tiles of 128 rows
PAIRS = list(combinations(range(F), 2))
P = len(PAIRS)            # 325
OUT_COLS = P * D          # 20800

N_PAIRS = [F - 1 - i for i in range(F - 1)]           # pairs with left field i
P_START = [sum(N_PAIRS[:i]) for i in range(F - 1)]    # first pair index of field i
FIELD_START = [P_START[i] * D for i in range(F - 1)]  # output col where field i begins
FIELD_END = [FIELD_START[i] + N_PAIRS[i] * D for i in range(F - 1)]

XNW = F * D               # xn cols per batch tile                           = 1664

# PSUM group grid per batch tile: (c0, c1, class).  FIELD-ALIGNED groups of
# <=2048 f32 cols (one pool slot = 4 banks; 2 bufs = whole PSUM).
#   V: DVE mul straight from PSUM (fp32, 1x) -- no ACT involvement.
#   A: ACT drain (f32 PSUM -> bf16 stage) + DVE in-place mul at 2x.
# GPSIMD is NOT used for muls: measured on HW, GpSimd tensor_tensor and DVE
# tensor_tensor serialize on the shared SBUF port pair (the blocked op stalls
# for the other's full duration), so GPSIMD adds no elementwise throughput.
# V groups are interleaved between A groups so DVE and ACT stay concurrently
# busy through the tile; ~21% of cols are V, balancing DVE (1x V muls + 2x A
# muls) against ACT (1x drains).
# The PE row group of all matmuls in a group is the GROUP index parity: two
# concurrent (different-row-group) matmuls writing the same PSUM bank is a
# fatal HW collision.  Same-group pieces (which may share a bank at interior
# field boundaries) share a row group -> serial -> safe; adjacent groups
# alternate -> dual-row-group concurrency across groups (separate pool slots
# -> disjoint banks).  Field-aligned groups mean each field has exactly one
# parity, so xt needs no duplication (1.7MB instead of 3.4).
GROUPS = [
    (0, 2048, 'A'), (2048, 4096, 'V'), (4096, 6144, 'A'),
    (6144, 8192, 'A'), (8192, 9856, 'V'),
    (9856, 11904, 'A'), (11904, 13952, 'A'), (13952, 16000, 'A'),
    (16000, 18048, 'A'), (18048, 20096, 'A'), (20096, 20800, 'V'),
]
HALF = 9856               # st0 covers cols [0, 9856), st1 [9856, 20800)

# Contiguous class runs (mul granularity: field pieces within a run).
RUNS = []
for (_g0, _g1, _cls) in GROUPS:
    if RUNS and RUNS[-1][2] == _cls and RUNS[-1][1] == _g0:
        RUNS[-1] = (RUNS[-1][0], _g1, _cls)
    else:
        RUNS.append((_g0, _g1, _cls))

# PSUM 512-col blocks (bank-aligned within each group's psum tile), carrying
# the group parity and the running per-parity W pack offset.  W's top half
# (partitions 0-63) holds even groups' columns in order, bottom half odd
# groups'.  Each field belongs to one group, hence one parity -> PAR[i].
# Block parity: the PE row group of every matmul is its 512-block's global
# index parity.  Same-bank pieces (same block) share a row group -> serial ->
# safe; adjacent blocks alternate -> dual-row-group concurrency everywhere,
# including inside large fields.  Costs xt duplication (all fields in both
# halves), which measured faster than field-parity variants that save the
# bytes but serialize each field's matmul stream.
BLOCKS = []               # (c0, c1, parity, w_off)
_offs = [0, 0]
_idx = 0
for (_g0, _g1, _cls) in GROUPS:
    _c = _g0
    while _c < _g1:
        _c1 = min(_c + 512, _g1)
        _par = _idx % 2
        BLOCKS.append((_c, _c1, _par, _offs[_par]))
        _offs[_par] += _c1 - _c
        _idx += 1
        _c = _c1
W_COLS = max(_offs)       # top half; the shorter half is padded
# W load chunk boundaries (w-offset space).  Finer early chunks let the
# first matmuls start ~6us in instead of ~12.
W_BOUNDS = [0, 1024, 3072, 6144, W_COLS]
# Output cols where a 512-block straddles a W chunk boundary in w-offset
# space: matmul pieces must split there so each rhs lives in one load tile.
W_EXTRA_SPLITS = []
for (_c0, _c1, _par, _boff) in BLOCKS:
    for _s in W_BOUNDS[1:-1]:
        if _boff < _s < _boff + (_c1 - _c0):
            W_EXTRA_SPLITS.append(_c0 + (_s - _boff))

# xt pack: every field in both halves (any block parity can use any field).
XT_OFF = {i: i * 128 for i in range(F)}
XTW = F * 128             # xt cols per batch tile (per half) = 3328


def _block_of(col):
    for b in BLOCKS:
        if b[0] <= col < b[1]:
            return b
    raise ValueError(col)

F32 = mybir.dt.float32
BF16 = mybir.dt.bfloat16


def _even_splits(c0, c1, n):
    step = -(-((c1 - c0) // n) // 64) * 64
    step = max(step, 64)
    out = []
    c = c0
    while c < c1:
        out.append((c, min(c + step, c1)))
        c += step
    return out


def _field_of(col):
    for i in range(F - 1):
        if FIELD_START[i] <= col < FIELD_END[i]:
            return i
    raise ValueError(col)


def _pieces(c0, c1, extra=()):
    """Split [c0, c1) at field starts and any extra boundaries.
    Returns list of (p0, p1, field)."""
    bounds = {c0, c1}
    bounds.update(s for s in FIELD_START if c0 < s < c1)
    bounds.update(e for e in extra if c0 < e < c1)
    bs = sorted(bounds)
    return [(a, b, _field_of(a)) for a, b in zip(bs, bs[1:])]


def _mm_pieces(g0, g1):
    """Matmul pieces: additionally split at 512-col PSUM bank boundaries
    (relative to the group base = block boundaries) and at w-load splits."""
    extra = set(range(g0 + 512, g1, 512))
    extra.update(W_EXTRA_SPLITS)
    return _pieces(g0, g1, extra)


def build_bass() -> bass.Bass:
    # Bisection flags (default off = full-featured kernel).
    no_gps = os.environ.get("K_NO_GPS", "0") == "1"       # gpsimd muls -> DVE
    no_inplace = os.environ.get("K_NO_INPLACE", "0") == "1"  # muls via cp tile
    swdge_loads = os.environ.get("K_SWDGE_LOADS", "0") == "1"  # loads on gpsimd
    wsplit = int(os.environ.get("K_WRITE_SPLIT", "2"))    # write DMAs per half
    nt_limit = int(os.environ.get("K_NT_LIMIT", str(NT)))  # batch tiles to run
    ngroups = int(os.environ.get("K_NGROUPS", str(len(GROUPS))))
    no_muls = os.environ.get("K_NO_MULS", "0") == "1"
    no_drains = os.environ.get("K_NO_DRAINS", "0") == "1"
    nc = bacc.Bacc()
    xn = nc.declare_dram_parameter("xn", [128, NT * XNW], BF16, isOutput=False)
    xt = nc.declare_dram_parameter("xt", [128, NT * XTW], BF16, isOutput=False)
    w = nc.declare_dram_parameter("w", [128, W_COLS], BF16, isOutput=False)
    out = nc.declare_dram_parameter("out", [BC, OUT_COLS], BF16, isOutput=True)

    with TileContext(nc) as tc:
        with (
            tc.tile_pool(name="consts", bufs=1) as consts,
            tc.tile_pool(name="stage", bufs=4) as stage_pool,
            tc.tile_pool(name="cp", bufs=2) as cp_pool,
            tc.tile_pool(name="psum", bufs=2, space="PSUM") as psum_pool,
        ):
            # Separate tile objects per load DMA keep dependency granularity
            # at the piece level.  Loads ride the SP HWDGE ring (its preamble
            # clears earliest), ordered by first use: tiny first chunks get
            # the first matmul going ~6us in; the output writes queue behind
            # but only start ~20us in, when all loads have drained.
            w_t = [consts.tile([128, b - a], BF16, tag=f"w{k}", name=f"w{k}")
                   for k, (a, b) in enumerate(zip(W_BOUNDS, W_BOUNDS[1:]))]
            xt00 = consts.tile([128, 256], BF16, tag="xt00", name="xt00")
            xt0r = consts.tile([128, XTW - 256], BF16, tag="xt0r", name="xt0r")
            xtr = consts.tile([128, (NT - 1) * XTW], BF16, tag="xtr", name="xtr")
            xn0 = consts.tile([128, XNW], BF16, tag="xn0", name="xn0")
            xnr = consts.tile([128, (NT - 1) * XNW], BF16, tag="xnr", name="xnr")

            # All loads on the SP ring in first-use order: the FIFO ring
            # drains them in priority order, so the critical early chunks
            # never share bandwidth with the later bulk loads.  (Spreading
            # issues across rings measured 20us WORSE -- the rings
            # round-robin at packet granularity, starving the early chunks.)
            ldeng = nc.sync if not swdge_loads else nc.gpsimd
            ldeng.dma_start(out=xt00[:], in_=xt[:, 0:256])
            ldeng.dma_start(out=w_t[0][:], in_=w[:, W_BOUNDS[0]:W_BOUNDS[1]])
            ldeng.dma_start(out=xn0[:], in_=xn[:, 0:XNW])
            ldeng.dma_start(out=xt0r[:], in_=xt[:, 256:XTW])
            ldeng.dma_start(out=w_t[1][:], in_=w[:, W_BOUNDS[1]:W_BOUNDS[2]])
            ldeng.dma_start(out=w_t[2][:], in_=w[:, W_BOUNDS[2]:W_BOUNDS[3]])
            ldeng.dma_start(out=w_t[3][:], in_=w[:, W_BOUNDS[3]:W_BOUNDS[4]])
            ldeng.dma_start(out=xtr[:], in_=xt[:, XTW:NT * XTW])
            ldeng.dma_start(out=xnr[:], in_=xn[:, XNW:NT * XNW])

            def xt_slice(t, i, r0):
                c = XT_OFF[i]
                if t == 0:
                    if c + 128 <= 256:
                        return xt00[r0:r0 + D, c:c + 128]
                    return xt0r[r0:r0 + D, c - 256:c - 256 + 128]
                c += (t - 1) * XTW
                return xtr[r0:r0 + D, c:c + 128]

            def w_slice(c0, c1):
                b0, b1, par, boff = _block_of(c0)
                assert c1 <= b1, (c0, c1, b0, b1)
                r0 = par * D
                wc = boff + (c0 - b0)
                n = c1 - c0
                for k in range(len(W_BOUNDS) - 1):
                    if wc + n <= W_BOUNDS[k + 1]:
                        assert wc >= W_BOUNDS[k], (c0, c1, wc)
                        wk = wc - W_BOUNDS[k]
                        return r0, w_t[k][r0:r0 + D, wk:wk + n]
                raise AssertionError((c0, c1, wc))

            def xn_slice(t, i, c0, c1):
                c = (i + 1) * D + (c0 - FIELD_START[i])
                if t > 0:
                    c += (t - 1) * XNW
                src = xn0 if t == 0 else xnr
                return src[:, c:c + (c1 - c0)]

            for t in range(nt_limit):
                st0 = stage_pool.tile([128, HALF], BF16, tag="stage",
                                      name=f"st{t}a")
                st1 = stage_pool.tile([128, OUT_COLS - HALF], BF16, tag="stage",
                                      name=f"st{t}b")

                def st_slice(c0, c1):
                    if c0 >= HALF:
                        return st1[:, c0 - HALF:c1 - HALF]
                    assert c1 <= HALF
                    return st0[:, c0:c1]

                if no_muls:
                    nc.vector.memset(st0[:], 0.0)
                    nc.vector.memset(st1[:], 0.0)

                # A/G-run mul pieces not yet emitted, per run index.
                pending = {}
                cp_tiles = {}
                for ri, (r0_, r1_, rcls) in enumerate(RUNS):
                    if rcls != 'V':
                        pending[ri] = _pieces(r0_, r1_)
                        if no_inplace:
                            cp_tiles[ri] = cp_pool.tile(
                                [128, r1_ - r0_], BF16, tag="cp",
                                name=f"cp{t}_{ri}")

                gskip = int(os.environ.get("K_GSKIP", "0"))
                mm_filter = os.environ.get("K_MM_FILTER")
                if mm_filter is not None:
                    mm_filter = {int(v) for v in mm_filter.split(",")}
                mm_idx = 0
                for (g0, g1, gcls) in GROUPS[gskip:ngroups]:
                    ps = psum_pool.tile([128, g1 - g0], F32, tag="ps",
                                        name=f"ps{t}_{g0}")
                    for (c0, c1, i) in _mm_pieces(g0, g1):
                        mm_idx += 1
                        if mm_filter is not None and (mm_idx - 1) not in mm_filter:
                            continue
                        r0, rhs = w_slice(c0, c1)
                        nc.tensor.matmul(
                            ps[:, c0 - g0:c1 - g0],
                            xt_slice(t, i, r0),
                            rhs,
                            start=True, stop=True,
                        )
                    if gcls == 'V':
                        for (c0, c1, i) in _pieces(g0, g1):
                            if no_muls:
                                break
                            nc.vector.tensor_mul(
                                st_slice(c0, c1),
                                ps[:, c0 - g0:c1 - g0],
                                xn_slice(t, i, c0, c1),
                            )
                    else:
                        ri = next(k for k, (a, b, cl) in enumerate(RUNS)
                                  if a <= g0 < b)
                        run0 = RUNS[ri][0]
                        # One big ACT drain (f32 PSUM -> bf16, cast).
                        if no_inplace:
                            drain_dst = cp_tiles[ri][:, g0 - run0:g1 - run0]
                        else:
                            drain_dst = st_slice(g0, g1)
                        if not no_drains:
                            nc.scalar.copy(out=drain_dst, in_=ps[:])
                        # Emit muls for run pieces fully drained now.
                        eng = nc.vector if (gcls == 'A' or no_gps) else nc.gpsimd
                        done = [pc for pc in pending[ri] if pc[1] <= g1]
                        for (c0, c1, i) in done:
                            pending[ri].remove((c0, c1, i))
                            if no_muls:
                                continue
                            if no_inplace:
                                msrc = cp_tiles[ri][:, c0 - run0:c1 - run0]
                            else:
                                msrc = st_slice(c0, c1)
                            eng.tensor_mul(
                                st_slice(c0, c1),
                                msrc,
                                xn_slice(t, i, c0, c1),
                            )
                    if g1 == HALF:
                        for (a, b) in _even_splits(0, HALF, wsplit):
                            nc.sync.dma_start(
                                out=out[t * 128:(t + 1) * 128, a:b],
                                in_=st0[:, a:b],
                            )
                if ngroups >= len(GROUPS):
                    assert all(not v for v in pending.values())
                    # Last tile: finer writes so the final transfer starts
                    # as early as possible (it is the kernel's tail).
                    ws = wsplit * 2 if t == NT - 1 else wsplit
                    for (a, b) in _even_splits(HALF, OUT_COLS, ws):
                        nc.sync.dma_start(
                            out=out[t * 128:(t + 1) * 128, a:b],
                            in_=st1[:, a - HALF:b - HALF],
                        )
    nc.compile()
    return nc


def prep_inputs(x: np.ndarray, W: np.ndarray):
    """Full inputs -> per-core in_maps with block-parity-packed bf16 layouts."""
    x = np.ascontiguousarray(np.asarray(x, dtype=np.float32))
    W = np.ascontiguousarray(np.asarray(W, dtype=np.float32))
    wg = W.transpose(1, 0, 2).reshape(D, OUT_COLS)
    w_top = np.zeros((D, W_COLS), np.float32)
    w_bot = np.zeros((D, W_COLS), np.float32)
    for (c0, c1, par, boff) in BLOCKS:
        dst = w_top if par == 0 else w_bot
        dst[:, boff:boff + (c1 - c0)] = wg[:, c0:c1]
    w_p = np.ascontiguousarray(
        np.concatenate([w_top, w_bot], axis=0).astype(ml_dtypes.bfloat16)
    )
    in_maps = []
    for c in range(NCORES):
        xc = x[:, c * BC:(c + 1) * BC, :]                       # [26, 512, 64]
        xn_p = np.ascontiguousarray(
            xc.reshape(F, NT, 128, D).transpose(2, 1, 0, 3)
            .reshape(128, NT * XNW).astype(ml_dtypes.bfloat16)
        )
        # xtd[d, t*XTW + f*128 + r] = xc[f, t*128+r, d], duplicated to both
        # partition halves so any block parity can use any field.
        xtd = (xc.reshape(F, NT, 128, D).transpose(3, 1, 0, 2)
               .reshape(D, NT * XTW))
        xt_p = np.ascontiguousarray(
            np.concatenate([xtd, xtd], axis=0).astype(ml_dtypes.bfloat16)
        )
        in_maps.append({"xn": xn_p, "xt": xt_p, "w": w_p})
    return in_maps


_CACHED_NC = None


def kernel(x: np.ndarray, W: np.ndarray) -> np.ndarray:
    global _CACHED_NC
    from concourse.bass_utils import run_bass_kernel_spmd

    if _CACHED_NC is None:
        _CACHED_NC = build_bass()
    in_maps = prep_inputs(x, W)
    res = run_bass_kernel_spmd(_CACHED_NC, in_maps, list(range(NCORES)))
    shards = [
        np.asarray(res.results[c]["out"]).astype(np.float32) for c in range(NCORES)
    ]
    return np.concatenate(shards, axis=0)
